# revision 1
# baseline (speedup 1.0000x reference)
"""Biaffine label attention kernel for 8 trn2 NeuronCores.

out[b, l, i, j] = (head[b] @ W_head.T)[i, l] + (dep[b] @ W_dep.T)[j, l] + bias[l]

with head/dep: [8, 512, 512] f32, label_W: [64, 1024], label_b: [64],
out: [8, 64, 512, 512] f32 (512 MB).

Sharding: data-parallel over batch; core b computes the contiguous 64 MB
slice out[b].  The kernel is output-write bound (~419 GB/s per core observed),
so the device program keeps the output DMAs maximally efficient (2 MB
dma_starts with 8 KB contiguous DRAM runs) while TensorE / ScalarE / VectorE
generate tiles well under the DMA rate:

  - Inputs arrive as 4 packed arrays (dma_start issue costs ~0.63 us each on
    the sequencer, so many small loads are issue-rate-limited), with all
    matmul operands pre-split on the host into bf16 hi+lo pairs: bf16
    cross-term matmuls accumulating in fp32 PSUM run ~8x faster than fp32
    matmuls on the PE while keeping ~1e-5 accuracy.
  - TensorE: a short HAM warm-up burst, then d' = dep@Wd^T + b and
    h = head@Wh^T (computed [l, i] and flipped into the swizzled [i, l]
    layout with four full-rate strided PE transposes), then one K=128
    selection-mask matmul per label over a [hi; lo] stacked d' tile to
    broadcast d'[l, :] across 128 partitions into PSUM (the mask constant
    avoids a ~5 us flatten-DMA round-trip on the critical path).
  - VectorE + ScalarE: 4 per-partition-scalar adds per label, reading the
    broadcast PSUM tile: out_tile[p, c*512 + j] = d'[l, j] + h[l, 4p + c].
  - Output: the first 4 labels ship as 1 MB DMAs (earliest first bytes),
    then one 2 MB HWDGE DMA per label pair (partition p holds rows
    4p..4p+3 -> 8 KB contiguous runs in DRAM).

Measured on 8 axon trn2 cores: ~197 us HW exec in the machine's light
power-throttle state (best 196,717 ns), ~220 us when heavily throttled
(throttle util-limit ~0.47 active ~88% of the run; not controllable from
the kernel), rel err ~4e-6.  Output data starts flowing ~30 us in; steady
output DMA runs at ~419 GB/s (96% of the 435 GB/s SBUF-fabric ceiling)
when lightly throttled.  The remaining overhead over the ~160 us output
roofline is the ~7 us fixed engine preamble, ~4.4 MB of input loads, the
serial d'/h prologue, and the final DMA queue drain.

Profiling notes for further iteration (from the per-instruction NTFF
timelines; engine-busy summaries hide all of these):
  - The chip throttle clamps engine clocks too: prologue matmuls run
    ~634 ns (1.2 GHz-class) even straight after a 6 us warm-up burst, so
    HAM warm-up cannot reach 2.4 GHz under throttle.  Prologue compute
    and DMA are both throttle-bound; structural work below ~2 us is
    unmeasurable against the +-30 us run-to-run throttle variance.
  - Tried and REVERTED (measurably worse): folding label_b into the d
    matmul group as K=1 terms (extending the dps PSUM lifetime into the
    DVE hi/lo chain regressed the schedule, +5 us of prologue gap);
    routing input DMAs over the scalar-engine HWDGE ring (steady output
    dropped ~419 -> ~400 GB/s); interleaving h-matmul accumulation
    groups kt-outer in one PSUM bank (illegal - concurrent groups per
    zero-region corrupt results; CoreSim catches it).
  - Next levers if continuing: per-c-block h_sw tiles so the first adds
    wait only on transpose block 0 (~0.5 us); half-label first warm DMA
    (~0.7 us to first byte); both sub-noise.
"""

import os
import sys
from contextlib import ExitStack

for _p in ("/opt/trn_rl_repo",):
    if os.path.isdir(_p) and _p not in sys.path:
        sys.path.insert(0, _p)

import numpy as np

import concourse.bass as bass
import concourse.bacc as bacc
import concourse.masks as masks
import concourse.tile as tile
from concourse import mybir
from concourse.bass_utils import run_bass_kernel_spmd

B = 8
S = 512
D = 512
L = 64
KT = D // 128  # contraction tiles
C = S // 128   # i-rows packed per partition
F32 = mybir.dt.float32

_NC_CACHE = None


def _build_nc():
    nc = bacc.Bacc(
        "TRN2", target_bir_lowering=False, debug=False, num_devices=B
    )
    BF16 = mybir.dt.bfloat16
    # Inputs packed into 4 arrays: dma_start issue costs ~0.63 us each on the
    # sequencer, so many small loads are issue-rate-limited (~200 GB/s).
    dep2d = nc.declare_dram_parameter("dep2", [128, 2 * KT * S], BF16, isOutput=False)
    head2d = nc.declare_dram_parameter("head2", [128, 2 * KT * S], BF16, isOutput=False)
    w4d = nc.declare_dram_parameter("w4", [128, 4 * KT * L], BF16, isOutput=False)
    biasv = nc.declare_dram_parameter("biasv", [L, 1], F32, isOutput=False)
    # seld[k, l*128 + p] = 1 iff k in {l, l+64}: lhsT windows that select the
    # (d_hi, d_lo) row pair for label l in the broadcast matmul.
    seld = nc.declare_dram_parameter("sel", [128, L * 128], BF16, isOutput=False)
    out = nc.declare_dram_parameter("out", [L, S, S], F32, isOutput=True)

    with tile.TileContext(nc) as tc, ExitStack() as ctx:
        const = ctx.enter_context(tc.tile_pool(name="const", bufs=1))
        psum_bc = ctx.enter_context(tc.tile_pool(name="psum_bc", bufs=5, space="PSUM"))
        psum_hd = ctx.enter_context(tc.tile_pool(name="psum_hd", bufs=1, space="PSUM"))
        out_pool = ctx.enter_context(tc.tile_pool(name="outp", bufs=6))

        # All loads share the sync HWDGE ring with the output writes — using
        # the scalar ring for inputs measurably slowed the steady-state
        # output stream (~400 vs ~419 GB/s), so everything stays on one ring.
        # d-path inputs first: the whole kernel is gated on d' being ready.
        w4 = const.tile([128, 4 * KT * L], BF16)
        nc.sync.dma_start(w4[:], w4d[:, :])
        bcol = const.tile([L, 1], F32)
        nc.sync.dma_start(bcol[:], biasv[:, :])
        dep2 = const.tile([128, 2 * KT * S], BF16)
        nc.sync.dma_start(dep2[:], dep2d[:, :])
        # sel is split so the first labels' mask windows land before head2 and
        # never gate the first broadcast; the bulk loads last, filling an
        # otherwise DMA-idle prologue window.  Separate tiles: Tile tracks
        # write deps per tile, so one tile with two writers would make the
        # first reader wait for both DMAs.
        NSEL_A = 16
        sel_a = const.tile([128, NSEL_A * 128], BF16)
        nc.sync.dma_start(sel_a[:], seld[:, : NSEL_A * 128])
        head2 = const.tile([128, 2 * KT * S], BF16)
        nc.sync.dma_start(head2[:], head2d[:, :])
        sel_b = const.tile([128, (L - NSEL_A) * 128], BF16)
        nc.sync.dma_start(sel_b[:], seld[:, NSEL_A * 128 :])

        def wslice(idx, kt):  # w4 packs [wdh | wdl | whh | whl], KT*L each
            base = idx * KT * L + kt * L
            return w4[:, base : base + L]

        def dslice(hi, kt):  # dep2 packs [hi | lo], KT*S each
            base = (0 if hi else KT * S) + kt * S
            return dep2[:, base : base + S]

        def hslice(hi, kt):
            base = (0 if hi else KT * S) + kt * S
            return head2[:, base : base + S]

        ones2 = const.tile([2, 128], BF16)
        nc.vector.memset(ones2[:], 1.0)
        wtile = const.tile([2, S], BF16)
        nc.vector.memset(wtile[:], 0.0)
        ident = const.tile([L, L], F32)
        masks.make_identity(nc, ident[:])

        # PE HAM warm-up: ~6 us of throwaway matmuls while the inputs load,
        # so the real prologue matmuls run at 2.4 GHz instead of 1.2 — sized
        # to keep the PE busy right up to dep2 landing (~14 us).
        for _ in range(10):
            wp = psum_bc.tile([128, S], F32, tag="bcp")
            nc.tensor.matmul(wp[:], ones2[:], wtile[:], start=True, stop=True)

        # d'[l, j] = sum_d dep[j, d] * W_dep[l, d] + b[l]   (l on partitions)
        # dep ~ dh + dl, wd ~ wh_ + wl_ in bf16; the three large cross terms
        # accumulate in fp32 PSUM at full PE rate (fp32 matmul is ~8x slower;
        # the dropped lo*lo term is ~2^-18 relative).
        dps = psum_hd.tile([L, S], F32)
        n_terms = 3 * KT
        ti = 0
        for kt in range(KT):
            for wi, dh in ((0, 1), (0, 0), (1, 1)):
                nc.tensor.matmul(
                    dps[:],
                    wslice(wi, kt),
                    dslice(dh, kt),
                    start=(ti == 0),
                    stop=(ti == n_terms - 1),
                )
                ti += 1
        d_sb = const.tile([L, S], F32)
        nc.scalar.add(d_sb[:], dps[:], bcol[:])

        # Split d' into bf16 hi + lo stacked in one [128, S] tile: hi on
        # partitions 0..63, lo on 64..127 (both legal engine AP bases).  The
        # per-label broadcast is then a K=128 matmul with a selection-mask
        # lhsT window — no flatten DMA round-trip on the critical path, and
        # full PE rate with ~fp32 precision (PSUM accumulates hi+lo in fp32).
        # Whole hi/lo chain on DVE: cross-engine hops cost ~0.3 us each in
        # semaphore latency and ScalarE is the prologue straggler.
        d_stack = const.tile([128, S], BF16)
        nc.vector.tensor_copy(d_stack[0:64, :], d_sb[:])
        d_hi32 = const.tile([L, S], F32)
        nc.vector.tensor_copy(d_hi32[:], d_stack[0:64, :])
        nc.vector.tensor_sub(d_stack[64:128, :], d_sb[:], d_hi32[:])

        # h[l, i] = sum_d head[i, d] * W_head[l, d], as bf16 hi/lo cross terms
        # (full PE rate; fp32 N=64 matmuls measured 2 HW passes = ~13 us and
        # gated the whole pipeline).  Result lands [l, i]; four strided PE
        # transposes then produce the swizzled [i, l] layout the adds need:
        # h_sw[p, c*L + l] = h[l, 4p + c].
        hps_li = psum_hd.tile([L, S], F32)
        ti = 0
        for kt in range(KT):
            for wi, hh in ((2, 1), (2, 0), (3, 1)):
                nc.tensor.matmul(
                    hps_li[:],
                    wslice(wi, kt),
                    hslice(hh, kt),
                    start=(ti == 0),
                    stop=(ti == 3 * KT - 1),
                )
                ti += 1
        # Both h copies run on DVE: ScalarE's queue (bias, hi32, then the
        # first tile's adds) is the prologue straggler, DVE runs ahead.
        h_li = const.tile([L, S], F32)
        nc.vector.tensor_copy(h_li[:], hps_li[:])
        h_li_str = h_li[:].rearrange("l (m c) -> l c m", c=C)
        hps_sw = psum_hd.tile([128, C * L], F32)
        for c in range(C):
            nc.tensor.transpose(
                hps_sw[:, c * L : (c + 1) * L], h_li_str[:, c, :], ident[:]
            )
        h_sw = const.tile([128, C * L], F32)
        nc.vector.tensor_copy(h_sw[:], hps_sw[:])

        # out[l, 4p + c, j] = d'[l, j] + h_sw[p, c*L + l]
        # The first few labels ship as single 1 MB DMAs so output bytes start
        # flowing as early as possible; the rest as 2 MB label-pair DMAs.
        N_WARM = 4
        out_r1 = out[:, :, :].rearrange("l (p c) j -> l p (c j)", c=C)
        out_r = out[:, :, :].rearrange("(lp m) (p c) j -> lp p m (c j)", m=2, c=C)

        def emit_label(l, ot, fbase):
            """Broadcast d'[l] and add h columns into ot[:, fbase:fbase+C*S]."""
            bcp = psum_bc.tile([128, S], F32)
            if l < NSEL_A:
                sel_win = sel_a[:, l * 128 : (l + 1) * 128]
            else:
                sel_win = sel_b[:, (l - NSEL_A) * 128 : (l - NSEL_A + 1) * 128]
            nc.tensor.matmul(
                bcp[:], sel_win, d_stack[:], start=True, stop=True
            )
            for c in range(C):
                scalar = h_sw[:, c * L + l : c * L + l + 1]
                dst = ot[:, fbase + c * S : fbase + (c + 1) * S]
                if c < 2:
                    nc.vector.tensor_scalar_add(dst, bcp[:], scalar)
                else:
                    nc.scalar.add(dst, bcp[:], scalar)

        warm_pool = ctx.enter_context(tc.tile_pool(name="warm", bufs=2))
        for l in range(N_WARM):
            ot = warm_pool.tile([128, C * S], F32)
            emit_label(l, ot, 0)
            nc.sync.dma_start(out_r1[l], ot[:])
        for lp in range(N_WARM // 2, L // 2):
            ot = out_pool.tile([128, 2 * C * S], F32)
            for m in range(2):
                emit_label(2 * lp + m, ot, m * C * S)
            nc.sync.dma_start(out_r[lp], ot[:])
    nc.compile()
    return nc


def _row_tile(a):
    """[D, F] -> [128, KT*F]: row d = kt*128 + p lands at [p, kt*F : (kt+1)*F]."""
    d, f = a.shape
    kt = d // 128
    return np.ascontiguousarray(
        a.reshape(kt, 128, f).transpose(1, 0, 2).reshape(128, kt * f)
    )


def _hi_lo(a):
    """f32 array -> (bf16 hi, bf16 lo) with a ~ hi + lo."""
    import ml_dtypes

    hi = a.astype(ml_dtypes.bfloat16)
    lo = (a - hi.astype(np.float32)).astype(ml_dtypes.bfloat16)
    return np.ascontiguousarray(hi), np.ascontiguousarray(lo)


def _prep_inputs(head, dep, label_W, label_b):
    head = np.asarray(head, dtype=np.float32)
    dep = np.asarray(dep, dtype=np.float32)
    label_W = np.asarray(label_W, dtype=np.float32)
    label_b = np.asarray(label_b, dtype=np.float32)

    whh, whl = _hi_lo(_row_tile(np.ascontiguousarray(label_W[:, :D].T)))
    wdh, wdl = _hi_lo(_row_tile(np.ascontiguousarray(label_W[:, D:].T)))
    w4 = np.ascontiguousarray(np.concatenate([wdh, wdl, whh, whl], axis=1))
    bias = np.ascontiguousarray(label_b.reshape(L, 1))

    import ml_dtypes

    sel = np.zeros((128, L * 128), dtype=ml_dtypes.bfloat16)
    for l in range(L):
        sel[l, l * 128 : (l + 1) * 128] = 1
        sel[l + 64, l * 128 : (l + 1) * 128] = 1

    in_maps = []
    for b in range(B):
        hth, htl = _hi_lo(_row_tile(np.ascontiguousarray(head[b].T)))
        dth, dtl = _hi_lo(_row_tile(np.ascontiguousarray(dep[b].T)))
        in_maps.append(
            {
                "head2": np.ascontiguousarray(np.concatenate([hth, htl], axis=1)),
                "dep2": np.ascontiguousarray(np.concatenate([dth, dtl], axis=1)),
                "w4": w4,
                "biasv": bias,
                "sel": sel,
            }
        )
    return in_maps


def _run(head, dep, label_W, label_b, trace=False, **trace_kwargs):
    global _NC_CACHE
    if _NC_CACHE is None:
        _NC_CACHE = _build_nc()
    in_maps = _prep_inputs(head, dep, label_W, label_b)
    res = run_bass_kernel_spmd(
        _NC_CACHE, in_maps, list(range(B)), trace=trace, **trace_kwargs
    )
    out = np.stack([res.results[i]["out"] for i in range(B)])
    return out, res


def kernel(head, dep, label_W, label_b):
    out, _ = _run(head, dep, label_W, label_b, trace=False)
    return out

